# revision 54
# baseline (speedup 1.0000x reference)
"""Trainium2 Bass kernel for a dense transformer block (B=2, T=2048, C=1024,
H=16, DFF=4096), distributed over 8 NeuronCores.

Sharding: 2 batch groups x 4-way query-block sharding. Core c handles batch
g=c//4 and query blocks {j, 7-j} (j=c%4) of 8 blocks of 256 rows. K/V are
computed per-core for the full batch (replicated; no collectives).

Key structure (v2):
- LN1 gamma/beta folded into the Q/K/V weights host-side; the kernel only
  computes x_hat = (x - m) * rstd row-major (per-partition scalars on DVE)
  and moves it feature-major with XBAR DMA transposes (no PE transposes,
  no scalar-engine copies).
- K-bias dropped (cancels in softmax); q/Wo/FFN biases ride rank-1
  ones-row matmuls accumulated into PSUM.
- Attention: scores [keys, q] as before, but AV is computed transposed
  (alpha as lhsT, V as rhs -> out [q, 65]) so the matmul free dim is 65
  and the softmax denominator lands as a per-partition scalar.
- exp is merged over two-bank PSUM spans; causal masks applied in two
  fused DVE multiplies per head.
- LN2 runs feature-major: column sums via ones-column matmuls, mean/rstd
  broadcast back with rank-1 matmuls, so x2 never goes row-major.
"""
import numpy as np
import ml_dtypes

import concourse.bass as bass
import concourse.mybir as mybir
import concourse.tile as tile
from concourse.vector_clock import ScopedClock
from concourse.bass_utils import run_bass_kernel_spmd

bf16 = ml_dtypes.bfloat16
fp8 = ml_dtypes.float8_e4m3fn
f32 = mybir.dt.float32
bt16 = mybir.dt.bfloat16
f8 = mybir.dt.float8e4
PM = mybir.MatmulPerfMode
FP8_PROJ = True
FP8_FFN = False
FP8_F2 = True
W8SCALE = 16.0
AF = mybir.ActivationFunctionType
OP = mybir.AluOpType

B, T, C, H, DH, DFF = 2, 2048, 1024, 16, 64, 4096
P = 128
QB = 256            # rows per query block
R = 512             # own query rows per core
RT = T + R          # rows through LN1 per core (full batch + own q rows)
CC = C // P         # 8 feature chunks
MM = DFF // P       # 32 ffn chunks
EPS = 1e-5


# ---------------------------------------------------------------------------
# The walrus build in this container rejects instructions with >1 sync wait.
# Tile's sem assignment can emit several on one instruction; split the excess
# onto same-engine NoOps placed immediately before.
def _patched_drain_and_barrier(self, tick_clock, wait_clock):
    nc = self.nc
    probe = nc.sync.nop(nofuse=True, hint="tail_wait_probe")
    wait_clock.add_sem_waits(probe.ins, ScopedClock({None: tick_clock.global_clock}))
    si = probe.ins.sync_info
    waits = list(si.on_wait) if si is not None else []
    if si is not None:
        si.on_wait = waits[:1]
    for w in waits[1:]:
        n2 = nc.sync.nop(nofuse=True, hint="tail_wait_split")
        n2.ins.sync_info = mybir.SyncInfo(on_wait=[w], on_update=[])
    nc.sync.drain()
    nc.all_engine_barrier()
    assert self.sems is not None
    popped = nc._tile_sem_poison_stack.pop()
    assert popped is self._sem_poison
    nc.clear_and_free_semaphores(list(self.sems.allocated().values()))
    nc.all_engine_barrier()


tile.TileContext._drain_and_barrier = _patched_drain_and_barrier

_MAX_WAITS = 1
_split_counter = [0]


def _split_sync_waits(nc):
    for fn in nc.m.functions:
        for bb in fn.blocks:
            new_insts = []
            for inst in bb.instructions:
                si = getattr(inst, "sync_info", None)
                lim = _MAX_WAITS
                if si is not None and si.on_wait and len(si.on_wait) > lim:
                    waits = list(si.on_wait)
                    keep = waits[-lim:]
                    excess = waits[:-lim]
                    for i in range(0, len(excess), _MAX_WAITS):
                        _split_counter[0] += 1
                        nop = mybir.InstNoOp(
                            name=f"I-wsplit-{_split_counter[0]}", ins=[], outs=[])
                        nop.engine = inst.engine
                        nop.sync_info = mybir.SyncInfo(
                            on_wait=excess[i:i + _MAX_WAITS], on_update=[])
                        new_insts.append(nop)
                    si.on_wait = keep
                new_insts.append(inst)
            bb.instructions = new_insts
# ---------------------------------------------------------------------------


class Ctx:
    pass


def _ln_tile(g, xt):
    """Row-major LN stats + normalize of xt [128, C] bf16 -> x_hat tile."""
    nc = g.nc
    st = g.stats.tile([P, 2, 6], f32, tag="bnst", name="bnst")
    xv = xt.rearrange("p (s d) -> p s d", s=2)
    for sg in range(2):
        nc.vector.bn_stats(out=st[:, sg, :], in_=xv[:, sg, :])
    mv = g.stats.tile([P, 2], f32, tag="bnmv", name="bnmv")
    nc.vector.bn_aggr(out=mv[:], in_=st[:])
    sq = g.stats.tile([P, 1], f32, tag="bnsq", name="bnsq")
    nc.scalar.activation(out=sq[:], in_=mv[:, 1:2], func=AF.Sqrt,
                         bias=g.eps_sb[:], scale=float(C) / (C - 1))
    rstd = g.stats.tile([P, 1], f32, tag="bnrstd", name="bnrstd")
    nc.vector.reciprocal(rstd[:], sq[:])
    xh = g.xhp.tile([P, C], bt16, tag="xhat", name="xhat")
    nc.vector.tensor_scalar(out=xh[:], in0=xt[:], scalar1=mv[:, 0:1],
                            scalar2=rstd[:], op0=OP.subtract, op1=OP.mult)
    return xh


def _phase_a(g):
    """LN1 + transposes + K/V projections (full batch) + Q projection."""
    nc = g.nc

    def ln_rb(rb):
        """Emit stats/normalize for one 512-row block; return the pending
        transpose list so the caller controls its SP-queue position."""
        outs = []
        for t in range(4):
            rt = rb * 4 + t
            xt = g.xio.tile([P, C], bt16, tag="xin", name="xin")
            nc.sync.dma_start(xt[:], g.xc[rt * P:(rt + 1) * P, :])
            outs.append(_ln_tile(g, xt))
        return outs

    def ln_TC(rb, outs):
        for t, xh in enumerate(outs):
            if rb < 4 and FP8_PROJ:
                # bf16 x1T for rb<4 is only an XBAR landing pad before the
                # fp8 cast -- use a small rotating scratch tile.
                dst = g.xscp.tile([P, CC, P], bt16, tag="xsc", name="xsc")[:]
            else:
                dst = g.x1T4[:, :, t * P:(t + 1) * P]
            nc.sync.dma_start_transpose(dst, xh[:])
            if FP8_PROJ:
                nc.gpsimd.tensor_copy(
                    out=g.x1f8[rb][:, :, t * P:(t + 1) * P], in_=dst)

    def k_rb(rb):
        for m in range(CC):
            pk = g.ps.tile([P, 512], f32, tag="ps", name="ps_k")
            if FP8_PROJ:
                for cp in range(4):
                    nc.tensor.matmul(pk[:], g.wks[:, cp, :, m * P:(m + 1) * P],
                                     g.x1f8[rb][:, 2 * cp:2 * cp + 2, :],
                                     start=(cp == 0), stop=(cp == 3),
                                     perf_mode=PM.DoubleRow)
                nc.scalar.mul(out=g.kT[m][rb][:, :], in_=pk[:],
                              mul=1.0 / W8SCALE)
            else:
                for c in range(CC):
                    nc.tensor.matmul(pk[:], g.wks[:, c, m * P:(m + 1) * P],
                                     g.x1T[rb][:, c, :],
                                     start=(c == 0), stop=(c == CC - 1))
                nc.scalar.copy(out=g.kT[m][rb][:, :], in_=pk[:])

    def v_rb(rb):
        for t in range(4):
            kt = rb * 4 + t
            for half in range(2):
                pv = g.ps.tile([P, 512], f32, tag="ps", name="ps_v")
                if FP8_PROJ:
                    for cp in range(4):
                        nc.tensor.matmul(
                            pv[:],
                            g.x1f8[rb][:, 2 * cp:2 * cp + 2,
                                       t * P:(t + 1) * P],
                            g.wvs[:, cp, :, half * 512:(half + 1) * 512],
                            start=(cp == 0), stop=(cp == 3),
                            perf_mode=PM.DoubleRow)
                else:
                    for c in range(CC):
                        nc.tensor.matmul(
                            pv[:], g.x1T[rb][:, c, t * P:(t + 1) * P],
                            g.wvs[:, c, half * 512:(half + 1) * 512],
                            start=(c == 0), stop=(c == CC - 1))
                sc = (1.0 / W8SCALE) if FP8_PROJ else 1.0
                nc.scalar.mul(
                    out=g.vv[kt][:, half * 8:(half + 1) * 8, 0:DH],
                    in_=pv.rearrange("p (h d) -> p h d", h=8), mul=sc)

    for kt in range(T // P):
        nc.gpsimd.memset(g.vv[kt][:, :, DH:DH + 1], 1.0)

    o0 = ln_rb(0)
    nc.sync.dma_start(g.wks[:], g.wk)
    o1 = ln_rb(1)
    ln_TC(0, o0)
    nc.sync.dma_start(g.wvs[:], g.wv)
    o2 = ln_rb(2)
    ln_TC(1, o1)
    nc.sync.dma_start(g.wqs[:], g.wq)
    o3 = ln_rb(3)
    ln_TC(2, o2)
    o4 = ln_rb(4)
    ln_TC(3, o3)
    ln_TC(4, o4)
    for rb in range(4):
        k_rb(rb)
        v_rb(rb)
    # Q projection for own rows (x1T[4]), bias via rank-1 matmul
    nc.sync.dma_start(g.bq_sb[:], g.bqv)
    for m in range(CC):
        pq = g.ps.tile([P, 512], f32, tag="ps", name="ps_q")
        if FP8_PROJ:
            for cp in range(4):
                nc.tensor.matmul(pq[:], g.wqs[:, cp, :, m * P:(m + 1) * P],
                                 g.x1f8[4][:, 2 * cp:2 * cp + 2, :],
                                 start=(cp == 0), stop=False,
                                 perf_mode=PM.DoubleRow)
        else:
            for c in range(CC):
                nc.tensor.matmul(pq[:], g.wqs[:, c, m * P:(m + 1) * P],
                                 g.x1T[4][:, c, :], start=(c == 0), stop=False)
        nc.tensor.matmul(pq[:], g.bq_sb[0:1, m * P:(m + 1) * P], g.ones512[:],
                         start=False, stop=True)
        sc = (1.0 / W8SCALE) if FP8_PROJ else 1.0
        nc.scalar.mul(out=g.qT[m][:, :], in_=pq[:], mul=sc)


def _phase_b(g):
    """Attention. Scores [keys, q] with fused early/late columns; exp merged
    over 2-bank PSUM spans; AV transposed (alpha as lhsT) with the softmax
    denominator as column 64; per-(pair, qc) XBAR transpose into hcat."""
    nc, tc = g.nc, g.tc
    with tc.tile_pool(name="apl", bufs=3) as apl, \
         tc.tile_pool(name="hqp", bufs=2) as hqp, \
         tc.tile_pool(name="scp", bufs=3, space="PSUM") as scp, \
         tc.tile_pool(name="mkp", bufs=1) as mkp, \
         tc.tile_pool(name="avp", bufs=2, space="PSUM") as avp:
        g.masks_sb = mkp.tile([P, 16, QB], bt16, tag="mask", name="mask")
        nc.sync.dma_start(g.masks_sb[:], g.masks)
        for pair in range(CC):
            if pair == 1:
                nc.sync.dma_start(g.wos[:], g.wo)
                nc.sync.dma_start(g.b1s[:], g.b1v.rearrange("c p -> p c"))
                nc.sync.dma_start(g.bo_sb[:], g.bov)
                nc.sync.dma_start(g.b2_sb[:], g.b2v)
                nc.sync.dma_start(g.g1s[:], g.g1v.rearrange("c p -> p c"))
                nc.sync.dma_start(g.g2s[:], g.g2v.rearrange("c p -> p c"))
            hqs = [hqp.tile([P, P], bt16, tag=f"hq{qc}", name=f"hq{qc}")
                   for qc in range(4)]
            for hl in range(2):
                h = 2 * pair + hl
                hs = slice(hl * DH, (hl + 1) * DH)
                aA = apl.tile([P, 8, 512], bt16, tag="aA", name="aA")
                aB = apl.tile([P, 8, QB], bt16, tag="aB", name="aB")
                # kc 0-7: 512 cols (early block 0:256, late 256:512)
                for grp in range(4):
                    psc = scp.tile([P, 1024], f32, tag="sc", name="ps_s")
                    for i in range(2):
                        kc = grp * 2 + i
                        rb, k0 = kc // 4, (kc % 4) * P
                        nc.tensor.matmul(
                            psc[:, i * 512:(i + 1) * 512],
                            g.kT[pair][rb][hs, k0:k0 + P], g.qT[pair][hs, :],
                            start=True, stop=True, skip_group_check=True)
                    nc.scalar.activation(out=aA[:, grp * 2:grp * 2 + 2, :],
                                         in_=psc.rearrange("p (a b) -> p a b",
                                                           a=2),
                                         func=AF.Exp)
                # kc 8-15: 256 cols (late block only)
                for grp in range(2):
                    psc = scp.tile([P, 1024], f32, tag="sc", name="ps_sl")
                    for i in range(4):
                        kc = 8 + grp * 4 + i
                        rb, k0 = kc // 4, (kc % 4) * P
                        nc.tensor.matmul(
                            psc[:, i * QB:(i + 1) * QB],
                            g.kT[pair][rb][hs, k0:k0 + P],
                            g.qT[pair][hs, QB:512],
                            start=True, stop=True, skip_group_check=True)
                    nc.scalar.activation(out=aB[:, grp * 4:grp * 4 + 4, :],
                                         in_=psc.rearrange("p (a b) -> p a b",
                                                           a=4),
                                         func=AF.Exp)
                # causal masks: early cols of aA (kc 0-7), all of aB (kc 8-15)
                nc.vector.tensor_mul(aA[:, :, 0:QB], aA[:, :, 0:QB],
                                     g.masks_sb[:, 0:8, :])
                nc.vector.tensor_mul(aB[:, :, :], aB[:, :, :],
                                     g.masks_sb[:, 8:16, :])
                # AV transposed: out [q, 65]; col 64 = softmax denominator
                pav = avp.tile([P, 4, 65], f32, tag="av", name="ps_av")
                for qc in range(4):
                    kcs = range(8) if qc < 2 else range(16)
                    last = kcs[-1]
                    for kc in kcs:
                        if kc < 8:
                            al = aA[:, kc, qc * P:(qc + 1) * P]
                        else:
                            al = aB[:, kc - 8, (qc - 2) * P:(qc - 1) * P]
                        nc.tensor.matmul(pav[:, qc, :], al, g.vv[kc][:, h, :],
                                         start=(kc == 0), stop=(kc == last),
                                         skip_group_check=True)
                for qc in range(4):
                    rr = g.stats.tile([P, 1], f32, tag="rr", name="rr")
                    nc.vector.reciprocal(rr[:], pav[:, qc, DH:DH + 1])
                    nc.vector.tensor_scalar(
                        out=hqs[qc][:, hl * DH:(hl + 1) * DH],
                        in0=pav[:, qc, 0:DH], scalar1=rr[:], scalar2=None,
                        op0=OP.mult)
            for qc in range(4):
                nc.sync.dma_start_transpose(
                    g.hcat[pair][:, qc * P:(qc + 1) * P], hqs[qc][:])


def _phase_cd(g):
    """Wo + residual; feature-major LN2; FFN; output."""
    nc, tc = g.nc, g.tc
    with tc.tile_pool(name="cdp", bufs=1) as cdp, \
         tc.tile_pool(name="sqp", bufs=3) as sqp:
        x2T = [cdp.tile([P, 512], f32, tag=f"x2T{m}", name=f"x2T{m}")
               for m in range(CC)]
        x2b = [cdp.tile([P, 512], bt16, tag=f"x2b{m}", name=f"x2b{m}")
               for m in range(CC)]
        # Wo with c outermost so the accumulation starts as soon as the first
        # hcat chunks are ready (the last chunk arrives late from phase B).
        with tc.tile_pool(name="wops", bufs=1, space="PSUM") as wops:
            pa = [wops.tile([P, 512], f32, tag=f"pso{m}", name=f"pso{m}")
                  for m in range(CC)]
            for c in range(CC):
                for m in range(CC):
                    nc.tensor.matmul(pa[m][:], g.wos[:, c, m * P:(m + 1) * P],
                                     g.hcat[c][:, :], start=(c == 0),
                                     stop=False)
            for m in range(CC):
                nc.tensor.matmul(pa[m][:], g.bo_sb[0:1, m * P:(m + 1) * P],
                                 g.ones512[:], start=False, stop=True)
                nc.vector.scalar_tensor_tensor(
                    out=x2T[m][:], in0=g.x1T4[:, m, :],
                    scalar=g.g1s[:, m:m + 1], in1=pa[m][:],
                    op0=OP.mult, op1=OP.add)
                nc.vector.tensor_copy(out=x2b[m][:], in_=x2T[m][:])

        # LN2, feature-major: column sums of x2 and x2^2 via ones-column MMs
        ln2_ffn(g, tc, nc, cdp, sqp, x2T, x2b)


def ln2_ffn(g, tc, nc, cdp, sqp, x2T, x2b):
    with tc.tile_pool(name="sps", bufs=1, space="PSUM") as sps, \
         tc.tile_pool(name="cps", bufs=4, space="PSUM") as cps:
        s1 = sps.tile([1, 512], f32, tag="s1", name="ps_s1")
        s2 = sps.tile([1, 512], f32, tag="s2", name="ps_s2")
        for m in range(CC):
            nc.tensor.matmul(s1[:], g.onescol[:], x2b[m][:],
                             start=(m == 0), stop=(m == CC - 1))
        for m in range(CC):
            sqb = sqp.tile([P, 512], bt16, tag="sqb", name="sqb")
            nc.vector.tensor_mul(sqb[:], x2b[m][:], x2b[m][:])
            nc.tensor.matmul(s2[:], g.onescol[:], sqb[:],
                             start=(m == 0), stop=(m == CC - 1))
        s1c = cdp.tile([1, 512], f32, tag="s1c", name="s1c")
        nc.vector.tensor_copy(out=s1c[:], in_=s1[:])
        t1 = cdp.tile([1, 512], f32, tag="t1", name="t1")
        nc.vector.tensor_mul(t1[:], s1c[:], s1c[:])
        t2 = cdp.tile([1, 512], f32, tag="t2", name="t2")
        nc.vector.scalar_tensor_tensor(out=t2[:], in0=t1[:],
                                       scalar=-1.0 / C, in1=s2[:],
                                       op0=OP.mult, op1=OP.add)
        sqr = cdp.tile([1, 512], f32, tag="sqr", name="sqr")
        nc.scalar.activation(out=sqr[:], in_=t2[:], func=AF.Sqrt,
                             bias=g.eps1[:], scale=1.0 / (C - 1))
        rstd1 = cdp.tile([1, 512], f32, tag="rstd1", name="rstd1")
        nc.vector.reciprocal(rstd1[:], sqr[:])
        mb = cdp.tile([1, 512], bt16, tag="mb", name="mb")
        nc.vector.tensor_scalar(out=mb[:], in0=s1c[:], scalar1=1.0 / C,
                                scalar2=None, op0=OP.mult)
        rb_ = cdp.tile([1, 512], bt16, tag="rb_", name="rb_")
        nc.vector.tensor_copy(out=rb_[:], in_=rstd1[:])
        pm = sps.tile([P, 512], f32, tag="pm", name="ps_pm")
        nc.tensor.matmul(pm[:], g.ones1r[:], mb[:], start=True, stop=True)
        pr = sps.tile([P, 512], f32, tag="pr", name="ps_pr")
        nc.tensor.matmul(pr[:], g.ones1r[:], rb_[:], start=True, stop=True)

        x3f = [cdp.tile([P, 512], f32, tag=f"x3f{m}", name=f"x3f{m}")
               for m in range(CC)]
        if FP8_FFN:
            x3h8 = cdp.tile([P, CC, 512], f8, tag="x3h8", name="x3h8")
        else:
            x3h = [cdp.tile([P, 512], bt16, tag=f"x3h{m}", name=f"x3h{m}")
                   for m in range(CC)]
        if FP8_FFN or FP8_F2:
            x3g = [cdp.tile([P, 512], f32, tag=f"x3g{m}", name=f"x3g{m}")
                   for m in range(CC)]
        for m in range(CC):
            tm = sqp.tile([P, 512], f32, tag="tm", name="tm")
            nc.vector.tensor_sub(tm[:], x2T[m][:], pm[:])
            nc.vector.tensor_mul(x3f[m][:], tm[:], pr[:])
            if FP8_FFN:
                nc.vector.tensor_copy(out=x3h8[:, m, :], in_=x3f[m][:])
            else:
                nc.vector.tensor_copy(out=x3h[m][:], in_=x3f[m][:])
            if FP8_FFN or FP8_F2:
                nc.vector.tensor_scalar(out=x3g[m][:], in0=x3f[m][:],
                                        scalar1=g.g2s[:, m:m + 1],
                                        scalar2=None, op0=OP.mult)

        # FFN (w1/w2 pools co-opened so the w2 stream doesn't serialize on
        # the w1 pool's SBUF space being freed)
        if FP8_FFN or FP8_F2:
            h18 = cdp.tile([P, MM, 512], f8, tag="h18", name="h18")
        else:
            h1 = [cdp.tile([P, 512], bt16, tag=f"h1_{m}", name=f"h1_{m}")
                  for m in range(MM)]
        with tc.tile_pool(name="w1p", bufs=4) as w1p, \
             tc.tile_pool(name="w2p", bufs=2) as w2p, \
             tc.tile_pool(name="otp", bufs=2) as otp:
            for m in range(MM):
                p1 = cps.tile([P, 512], f32, tag="ps", name="ps_f1")
                if FP8_FFN:
                    w1m = w1p.tile([P, 4, 2, P], f8, tag="w1m", name="w1m")
                    nc.sync.dma_start(w1m[:], g.w1[m])
                    for cp in range(4):
                        nc.tensor.matmul(p1[:], w1m[:, cp, :, :],
                                         x3h8[:, 2 * cp:2 * cp + 2, :],
                                         start=(cp == 0), stop=(cp == 3),
                                         perf_mode=PM.DoubleRow)
                    nc.scalar.activation(out=h18[:, m, :], in_=p1[:],
                                         func=AF.Gelu,
                                         bias=g.b1s[:, m:m + 1],
                                         scale=1.0 / W8SCALE)
                else:
                    w1m = w1p.tile([P, CC, P], bt16, tag="w1m", name="w1m")
                    nc.sync.dma_start(w1m[:], g.w1[m])
                    for c in range(CC):
                        nc.tensor.matmul(p1[:], w1m[:, c, :], x3h[c][:],
                                         start=(c == 0), stop=(c == CC - 1))
                    h1out = h18[:, m, :] if FP8_F2 else h1[m][:]
                    nc.scalar.activation(out=h1out, in_=p1[:], func=AF.Gelu,
                                         bias=g.b1s[:, m:m + 1], scale=1.0)
            for oc in range(CC):
                p2 = cps.tile([P, 512], f32, tag="ps", name="ps_f2")
                if FP8_F2:
                    w2m = w2p.tile([P, 16, 2, P], f8, tag="w2m", name="w2m")
                    nc.sync.dma_start(w2m[:], g.w2[oc])
                    w2rm = w2p.tile([P, 16, 2, P], f8, tag="w2rm",
                                    name="w2rm")
                    nc.sync.dma_start(w2rm[:], g.w2r[oc])
                    for kp in range(16):
                        nc.tensor.matmul(p2[:], w2m[:, kp, :, :],
                                         h18[:, 2 * kp:2 * kp + 2, :],
                                         start=(kp == 0), stop=False,
                                         perf_mode=PM.DoubleRow)
                    for kp in range(16):
                        nc.tensor.matmul(p2[:], w2rm[:, kp, :, :],
                                         h18[:, 2 * kp:2 * kp + 2, :],
                                         start=False, stop=False,
                                         perf_mode=PM.DoubleRow)
                elif FP8_FFN:
                    w2m = w2p.tile([P, 16, 2, P], f8, tag="w2m", name="w2m")
                    nc.sync.dma_start(w2m[:], g.w2[oc])
                    for kp in range(16):
                        nc.tensor.matmul(p2[:], w2m[:, kp, :, :],
                                         h18[:, 2 * kp:2 * kp + 2, :],
                                         start=(kp == 0), stop=False,
                                         perf_mode=PM.DoubleRow)
                else:
                    w2m = w2p.tile([P, MM, P], bt16, tag="w2m", name="w2m")
                    nc.sync.dma_start(w2m[:], g.w2[oc])
                    for k in range(MM):
                        nc.tensor.matmul(p2[:], w2m[:, k, :], h1[k][:],
                                         start=(k == 0), stop=False)
                nc.tensor.matmul(p2[:], g.b2_sb[0:1, oc * P:(oc + 1) * P],
                                 g.ones512[:], start=False, stop=True)
                ot = otp.tile([P, R], f32, tag="otile", name="otile")
                if FP8_FFN or FP8_F2:
                    nc.vector.scalar_tensor_tensor(
                        out=ot[:], in0=p2[:], scalar=1.0 / W8SCALE,
                        in1=x3g[oc][:], op0=OP.mult, op1=OP.add)
                else:
                    nc.vector.scalar_tensor_tensor(
                        out=ot[:], in0=x3f[oc][:], scalar=g.g2s[:, oc:oc + 1],
                        in1=p2[:], op0=OP.mult, op1=OP.add)
                nc.scalar.dma_start(g.out[oc], ot[:])


def build_kernel():
    nc = bass.Bass("TRN2", target_bir_lowering=False, num_devices=8)
    g = Ctx()
    g.nc = nc

    g.xc = nc.dram_tensor("xc", [RT, C], bt16, kind="ExternalInput").ap()
    wsh = ([P, 4, 2, C], f8) if FP8_PROJ else ([P, CC, C], bt16)
    g.wk = nc.dram_tensor("wk", *wsh, kind="ExternalInput").ap()
    g.wq = nc.dram_tensor("wq", *wsh, kind="ExternalInput").ap()
    g.wv = nc.dram_tensor("wv", *wsh, kind="ExternalInput").ap()
    g.wo = nc.dram_tensor("wo", [P, CC, C], bt16, kind="ExternalInput").ap()
    w1sh = ([P, 4, 2, P], f8) if FP8_FFN else ([P, CC, P], bt16)
    w2sh = ([P, 16, 2, P], f8) if (FP8_FFN or FP8_F2) else ([P, MM, P], bt16)
    g.w1 = [nc.dram_tensor(f"w1_{m}", *w1sh,
                           kind="ExternalInput").ap() for m in range(MM)]
    g.w2 = [nc.dram_tensor(f"w2_{o}", *w2sh,
                           kind="ExternalInput").ap() for o in range(CC)]
    if FP8_F2:
        g.w2r = [nc.dram_tensor(f"w2r_{o}", [P, 16, 2, P], f8,
                                kind="ExternalInput").ap() for o in range(CC)]
    g.masks = nc.dram_tensor("masks", [P, 16, QB], bt16,
                             kind="ExternalInput").ap()
    g.bqv = nc.dram_tensor("bqv", [1, C], bt16, kind="ExternalInput").ap()
    g.bov = nc.dram_tensor("bov", [1, C], bt16, kind="ExternalInput").ap()
    g.b2v = nc.dram_tensor("b2v", [1, C], bt16, kind="ExternalInput").ap()
    g.b1v = nc.dram_tensor("b1v", [MM, P], f32, kind="ExternalInput").ap()
    g.g1v = nc.dram_tensor("g1v", [CC, P], f32, kind="ExternalInput").ap()
    g.g2v = nc.dram_tensor("g2v", [CC, P], f32, kind="ExternalInput").ap()
    g.out = nc.dram_tensor("out", [CC, P, R], f32, kind="ExternalOutput").ap()

    with tile.TileContext(nc) as tc:
        g.tc = tc
        with tc.tile_pool(name="setup", bufs=1) as setup, \
             tc.tile_pool(name="stats", bufs=4) as stats, \
             tc.tile_pool(name="x1t4p", bufs=1) as x1t4p, \
             tc.tile_pool(name="hp", bufs=1) as hp, \
             tc.tile_pool(name="wop", bufs=1) as wop:
            g.stats = stats
            g.wos = wop.tile([P, CC, C], bt16, tag="wos", name="wos")

            g.eps_sb = setup.tile([P, 1], f32, tag="eps", name="eps")
            nc.vector.memset(g.eps_sb[:], EPS)
            g.eps1 = setup.tile([1, 1], f32, tag="eps1", name="eps1")
            nc.vector.memset(g.eps1[:], EPS)
            g.ones512 = setup.tile([1, 512], bt16, tag="ones512", name="ones512")
            nc.vector.memset(g.ones512[:], 1.0)
            g.onescol = setup.tile([P, 1], bt16, tag="onescol", name="onescol")
            nc.vector.memset(g.onescol[:], 1.0)
            g.ones1r = setup.tile([1, P], bt16, tag="ones1r", name="ones1r")
            nc.vector.memset(g.ones1r[:], 1.0)
            g.bq_sb = setup.tile([1, C], bt16, tag="bq", name="bq")
            g.bo_sb = setup.tile([1, C], bt16, tag="bo", name="bo")
            g.b2_sb = setup.tile([1, C], bt16, tag="b2", name="b2")
            g.b1s = setup.tile([P, MM], f32, tag="b1s", name="b1s")
            g.g1s = setup.tile([P, CC], f32, tag="g1s", name="g1s")
            g.g2s = setup.tile([P, CC], f32, tag="g2s", name="g2s")


            g.x1T4 = x1t4p.tile([P, CC, 512], bt16, tag="x1T4", name="x1T4")
            g.hcat = [hp.tile([P, 512], bt16, tag=f"hcat{c}", name=f"hcat{c}")
                      for c in range(CC)]

            with tc.tile_pool(name="kvp", bufs=1) as kvp:
                if not FP8_PROJ:
                    g.x1T = [kvp.tile([P, CC, 512], bt16, tag=f"x1T{rb}",
                                      name=f"x1T{rb}") for rb in range(4)]
                    g.x1T.append(g.x1T4)
                g.kT = [[kvp.tile([P, 512], bt16, tag=f"kT{m}_{rb}",
                                  name=f"kT{m}_{rb}") for rb in range(4)]
                        for m in range(CC)]
                g.vv = [kvp.tile([P, H, DH + 1], bt16, tag=f"vv{kt}",
                                 name=f"vv{kt}") for kt in range(T // P)]
                g.qT = [kvp.tile([P, 512], bt16, tag=f"qT{m}", name=f"qT{m}")
                        for m in range(CC)]
                with tc.tile_pool(name="ps", bufs=8, space="PSUM") as ps, \
                     tc.tile_pool(name="wap", bufs=1) as wap, \
                     tc.tile_pool(name="xio", bufs=8) as xio, \
                     tc.tile_pool(name="xhp", bufs=8) as xhp, \
                     tc.tile_pool(name="xscp", bufs=4) as xscp:
                    g.ps = ps
                    g.xio, g.xhp, g.xscp = xio, xhp, xscp
                    if FP8_PROJ:
                        g.wks = wap.tile([P, 4, 2, C], f8, tag="wks",
                                         name="wks")
                        g.wvs = wap.tile([P, 4, 2, C], f8, tag="wvs",
                                         name="wvs")
                        g.wqs = wap.tile([P, 4, 2, C], f8, tag="wqs",
                                         name="wqs")
                        g.x1f8 = [wap.tile([P, CC, 512], f8, tag=f"x1f8{rb}",
                                           name=f"x1f8{rb}")
                                  for rb in range(5)]
                    else:
                        g.wks = wap.tile([P, CC, C], bt16, tag="wks",
                                         name="wks")
                        g.wvs = wap.tile([P, CC, C], bt16, tag="wvs",
                                         name="wvs")
                        g.wqs = wap.tile([P, CC, C], bt16, tag="wqs",
                                         name="wqs")
                    _phase_a(g)
                _phase_b(g)
            _phase_cd(g)
    _split_sync_waits(nc)
    return nc


_NC_CACHE = None


def _get_nc():
    global _NC_CACHE
    if _NC_CACHE is None:
        _NC_CACHE = build_kernel()
    return _NC_CACHE


def _prep_shared(inputs):
    scale = DH ** -0.5
    g1 = np.asarray(inputs["gamma1"], np.float32).reshape(C)
    be1 = np.asarray(inputs["beta1"], np.float32).reshape(C)
    g2 = np.asarray(inputs["gamma2"], np.float32).reshape(C)
    be2 = np.asarray(inputs["beta2"], np.float32).reshape(C)
    Wq = np.asarray(inputs["Wq"], np.float32).transpose(1, 0, 2).reshape(C, C)
    Wk = np.asarray(inputs["Wk"], np.float32).transpose(1, 0, 2).reshape(C, C)
    Wv = np.asarray(inputs["Wv"], np.float32).transpose(1, 0, 2).reshape(C, C)
    Wo = np.asarray(inputs["Wo"], np.float32)
    W1 = np.asarray(inputs["W1"], np.float32)
    W2 = np.asarray(inputs["W2"], np.float32)
    bv_c = np.asarray(inputs["bv"], np.float32).reshape(C)
    bq_c = np.asarray(inputs["bq"], np.float32).reshape(C)
    bo_c = np.asarray(inputs["bo"], np.float32)
    b1_c = np.asarray(inputs["b1"], np.float32)
    b2_c = np.asarray(inputs["b2"], np.float32)

    def chunked(Wf):
        # [C, C] -> [P, CC, C] with W[c*128+p, f] at [p, c, f]
        return np.ascontiguousarray(
            Wf.reshape(CC, P, C).transpose(1, 0, 2)).astype(bf16)

    def dr_chunked(Wf):
        # [C, F] -> [P, C//256, 2, F] fp8 with 16*W[(2cp+ko)*128+p, f]
        F = Wf.shape[1]
        return np.ascontiguousarray(
            (W8SCALE * Wf).reshape(C // 256, 2, P, F).transpose(2, 0, 1, 3)
        ).astype(fp8)

    w1g = g2[:, None] * W1                      # gamma2 folded
    bqs = 1.0
    if FP8_PROJ:
        wq_h, wk_h, wv_h = (dr_chunked(g1[:, None] * Wq * scale),
                            dr_chunked(g1[:, None] * Wk),
                            dr_chunked(g1[:, None] * Wv))
        bqs = W8SCALE
    else:
        wq_h, wk_h, wv_h = (chunked(g1[:, None] * Wq * scale),
                            chunked(g1[:, None] * Wk),
                            chunked(g1[:, None] * Wv))
    b2s = W8SCALE if (FP8_FFN or FP8_F2) else 1.0
    shared = {
        "wq": wq_h,
        "wk": wk_h,
        "wv": wv_h,
        "wo": chunked(Wo),
        "bqv": (bq_c * scale * bqs).reshape(1, C).astype(bf16),
        "bov": (bo_c + bv_c @ Wo + be1).reshape(1, C).astype(bf16),
        "b2v": ((b2_c + be2) * b2s).reshape(1, C).astype(bf16),
        "b1v": (b1_c + be2 @ W1).reshape(MM, P).astype(np.float32),
        "g1v": g1.reshape(CC, P).astype(np.float32),
        "g2v": g2.reshape(CC, P).astype(np.float32),
    }
    for m in range(MM):
        blk = w1g[:, m * P:(m + 1) * P]
        if FP8_FFN:
            shared[f"w1_{m}"] = dr_chunked(blk)
        else:
            shared[f"w1_{m}"] = np.ascontiguousarray(
                blk.reshape(CC, P, P).transpose(1, 0, 2)).astype(bf16)
    for o in range(CC):
        blk = W2[:, o * P:(o + 1) * P]
        if FP8_F2:
            sc16 = W8SCALE * blk
            q = sc16.astype(fp8)
            r = sc16 - q.astype(np.float32)
            shared[f"w2_{o}"] = np.ascontiguousarray(
                q.astype(np.float32).reshape(16, 2, P, P).transpose(2, 0, 1, 3)
            ).astype(fp8)
            shared[f"w2r_{o}"] = np.ascontiguousarray(
                r.reshape(16, 2, P, P).transpose(2, 0, 1, 3)).astype(fp8)
        elif FP8_FFN:
            shared[f"w2_{o}"] = np.ascontiguousarray(
                (W8SCALE * blk).reshape(16, 2, P, P).transpose(2, 0, 1, 3)
            ).astype(fp8)
        else:
            shared[f"w2_{o}"] = np.ascontiguousarray(
                blk.reshape(MM, P, P).transpose(1, 0, 2)).astype(bf16)
    return shared


def _core_masks(j):
    """[128, 16, 256] bf16 0/1 masks. kc 0-7 mask the early block's columns
    (block j); kc 8-15 mask the late block's columns (block 7-j)."""
    out = np.zeros((P, 16, QB), np.float32)
    for kc in range(16):
        b = j if kc < 8 else 7 - j
        key = kc * P + np.arange(P)[:, None]          # [128, 1]
        qglob = b * QB + np.arange(QB)[None, :]       # [1, 256]
        out[:, kc, :] = (key <= qglob)
    return out.astype(bf16)


def _make_in_maps(inputs):
    x = np.asarray(inputs["x"], np.float32)
    shared = _prep_shared(inputs)
    in_maps = []
    for c in range(8):
        gg, j = c // 4, c % 4
        xb = x[gg]
        xq = np.concatenate([xb[j * QB:(j + 1) * QB],
                             xb[(7 - j) * QB:(8 - j) * QB]], 0)
        m = dict(shared)
        m["xc"] = np.ascontiguousarray(
            np.concatenate([xb, xq], 0)).astype(bf16)
        m["masks"] = _core_masks(j)
        in_maps.append(m)
    return in_maps


def _assemble(results):
    out = np.zeros((B, T, C), np.float32)
    for c in range(8):
        gg, j = c // 4, c % 4
        o = results[c]["out"].reshape(C, R).T  # [512, C] rows = 2 blocks
        out[gg, j * QB:(j + 1) * QB] = o[:QB]
        out[gg, (7 - j) * QB:(8 - j) * QB] = o[QB:]
    return out


def kernel(**inputs):
    in_maps = _make_in_maps(inputs)
    nc = _get_nc()
    res = run_bass_kernel_spmd(nc, in_maps, core_ids=list(range(8)))
    return _assemble(res.results)
